# revision 8
# baseline (speedup 1.0000x reference)
"""CommutatorConv2d kernel for Trainium2 (Bass/Tile), 8-core data-parallel.

Math: the reference's commutator/anticommutator conv reduces exactly to a
single-channel 3x3 conv on the channel-summed input:

    out[b] = T @ xs[b] @ A + Bm @ xs[b] @ T + bias,   xs = x.sum(axis=1)

where T is the 128x128 tridiagonal-ones matrix and A, Bm are tridiagonal
matrices built from K's column/row sums scaled by (lambda_c +/- lambda_a).

v3: everything bf16 (host casts once; harness gate is 2e-2, this lands
~4e-3): halves HBM traffic and runs the PE at 1 cycle/row. Layout
[H, B_loc, C, W] so a piece (channel slice) is one contiguous run per
partition; pieces are 16/12/4 channels -> 4KB/3KB/1KB DMA descriptors
(the HWDGE descriptor-generation rate, ~85M desc/s/queue, limits
throughput for small descriptors). Batch 1 + its stores ride the sync
ring; constants + batch 0 ride the scalar ring, so the two batches'
tails stagger naturally.

Fold: the PE folds each batch's leading 16 channels as 4 accumulating
N=512 identity matmuls (amortizes the PE's 173ns SBUF access latency);
ACT evacuates the [128,512] PSUM to bf16 mid-stream and the DVE finishes
it 512->128. The DVE (bf16 2x mode) trees the 12ch and 4ch pieces and
combines partials into xs. Sandwich per batch: uv = xs.T @ [T | BmT],
then out = uv1.T @ A + uv2.T @ T accumulated in PSUM, with ACT doing
PSUM evacuation and the fused bias add.
"""

import numpy as np

B, C, H, W = 16, 32, 128, 128
N_CORES = 8
B_LOC = B // N_CORES

PIECE_CH = (16, 12, 4)

_PROGRAM = None
LAST_RESULTS = None


def _build_program():
    import concourse.mybir as mybir
    from concourse import bacc
    from concourse.bass import MemorySpace
    from concourse.tile import TileContext

    bf16 = mybir.dt.bfloat16
    f32 = mybir.dt.float32
    nc = bacc.Bacc(
        "TRN2", target_bir_lowering=False, debug=False, num_devices=N_CORES
    )

    x_dram = nc.dram_tensor("x", (H, B_LOC, C, W), bf16, kind="ExternalInput")
    # fused constants: [A | T | BmT | I | bias-bits] as bf16 columns; the
    # last two bf16 columns hold the fp32 bias bit pattern (bitcast on chip)
    cm_dram = nc.dram_tensor("cmat", (H, 4 * W + 2), bf16, kind="ExternalInput")
    out_dram = nc.dram_tensor("out", (H, B_LOC, W), f32, kind="ExternalOutput")

    x_ap = x_dram.ap()
    out_ap = out_dram.ap()

    with TileContext(nc) as tc:
        with (
            tc.tile_pool(name="consts", bufs=1) as cpool,
            tc.tile_pool(name="xpool", bufs=1) as xpool,
            tc.tile_pool(name="spool", bufs=1) as spool,
            tc.tile_pool(name="psum", bufs=1, space=MemorySpace.PSUM) as ppool,
        ):
            cm_sb = cpool.tile([H, 4 * W + 2], bf16, tag="cm")
            a_sb = cm_sb[:, 0:W]
            t_sb = cm_sb[:, W : 2 * W]
            tbm_sb = cm_sb[:, W : 3 * W]  # [T | BmT]
            i_sb = cm_sb[:, 3 * W : 4 * W]
            bias_sb = cm_sb[:, 4 * W : 4 * W + 2].bitcast(f32)

            # constants ride the gpsimd (SWDGE) ring so both HWDGE rings
            # carry only x; their descriptor-generation budget is the
            # stream bottleneck (~15ns/descriptor/queue)
            nc.gpsimd.dma_start(out=cm_sb, in_=cm_dram.ap())
            tiles = {}
            for b, eng in ((1, nc.sync), (0, nc.scalar)):
                c0 = 0
                for p, nch in enumerate(PIECE_CH):
                    xq = xpool.tile([H, nch * W], bf16, tag=f"x{b}_{p}")
                    eng.dma_start(
                        out=xq.rearrange("h (c w) -> h c w", w=W),
                        in_=x_ap[:, b, c0 : c0 + nch, :],
                    )
                    tiles[(b, p)] = xq
                    c0 += nch

            # ---- PE fold of each batch's 16ch piece: 4 x N=512 quads ----
            fold_psum = {}
            for b in (1, 0):
                psum = ppool.tile([H, 4 * W], f32, tag=f"fold{b}")
                xq = tiles[(b, 0)]
                for q in range(4):
                    nc.tensor.matmul(
                        psum,
                        i_sb,
                        xq[:, q * 4 * W : (q + 1) * 4 * W],
                        start=(q == 0),
                        stop=(q == 3),
                        skip_group_check=True,
                    )
                fold_psum[b] = psum

            # ACT evacuates fold PSUMs to bf16 (mid-stream, hidden)
            p0_sb = {}
            for b in (1, 0):
                sb = spool.tile([H, 4 * W], bf16, tag=f"p0_{b}")
                nc.scalar.copy(sb, fold_psum[b])
                p0_sb[b] = sb

            # ---- DVE: finish 512->128, tree the 12ch + 4ch pieces ----
            def tree(ap_tile, nelem):
                # in-place halving tree down to W elems (nelem mult of 2W)
                n = nelem
                while n > W and n % 2 == 0 and (n // 2) % W == 0:
                    n //= 2
                    nc.vector.tensor_add(
                        ap_tile[:, :n], ap_tile[:, :n], ap_tile[:, n : 2 * n]
                    )
                # handle 3W leftover (12ch path: 1536->768->384 = 3W)
                if n == 3 * W:
                    nc.vector.tensor_add(
                        ap_tile[:, :W], ap_tile[:, :W], ap_tile[:, W : 2 * W]
                    )
                    nc.vector.tensor_add(
                        ap_tile[:, :W], ap_tile[:, :W], ap_tile[:, 2 * W : 3 * W]
                    )

            # DVE work in expected-ready order: 12ch trees first (arrive
            # mid-stream), then each batch's PE-partial finish + 4ch tail
            # tree + combines
            xs = {}
            tree(tiles[(1, 1)], 12 * W)
            tree(tiles[(0, 1)], 12 * W)
            for b in (1, 0):
                pp = p0_sb[b]
                tree(pp, 4 * W)  # finish PE partial: 512 -> 128
                t4 = tiles[(b, 2)]
                tree(t4, 4 * W)
                nc.vector.tensor_add(pp[:, :W], pp[:, :W], tiles[(b, 1)][:, :W])
                nc.vector.tensor_add(pp[:, :W], pp[:, :W], t4[:, :W])
                xs[b] = pp[:, :W]

            # ---- sandwich per batch (b1 first: its stream ends first);
            # stores split into half-partition DMAs across both rings so the
            # ~15ns/descriptor generation cost halves ----
            for b in (1, 0):
                uv_psum = ppool.tile([H, 2 * W], f32, tag=f"uv{b}p")
                nc.tensor.matmul(uv_psum, xs[b], tbm_sb, start=True, stop=True)
                uv_sb = spool.tile([H, 2 * W], bf16, tag=f"uv{b}")
                nc.scalar.copy(uv_sb, uv_psum)
                o_psum = ppool.tile([H, W], f32, tag=f"o{b}p")
                nc.tensor.matmul(
                    o_psum, uv_sb[:, 0:W], a_sb, start=True, stop=False,
                    skip_group_check=True,
                )
                nc.tensor.matmul(
                    o_psum, uv_sb[:, W : 2 * W], t_sb, start=False, stop=True,
                    skip_group_check=True,
                )
                o_sb = spool.tile([H, W], f32, tag=f"o{b}")
                nc.scalar.add(o_sb, o_psum, add=bias_sb)
                nc.sync.dma_start(
                    out=out_ap[0 : H // 2, b, :], in_=o_sb[0 : H // 2, :]
                )
                nc.scalar.dma_start(
                    out=out_ap[H // 2 : H, b, :], in_=o_sb[H // 2 : H, :]
                )

    nc.compile()
    return nc


def _get_program():
    global _PROGRAM
    if _PROGRAM is None:
        _PROGRAM = _build_program()
    return _PROGRAM


def _build_consts(K, bias, lambda_c, lambda_a):
    import ml_dtypes

    K = np.asarray(K, np.float32)
    lc = float(np.asarray(lambda_c))
    la = float(np.asarray(lambda_a))
    a = (lc + la) * K.sum(axis=0)  # column sums -> horizontal taps
    b = (la - lc) * K.sum(axis=1)  # row sums -> vertical taps
    eye = np.eye(H, dtype=np.float32)
    up = np.eye(H, k=1, dtype=np.float32)
    dn = np.eye(H, k=-1, dtype=np.float32)
    T = eye + up + dn
    A = a[1] * eye + a[0] * up + a[2] * dn
    Bm = b[1] * eye + b[2] * up + b[0] * dn
    cm = np.concatenate([A, T, Bm.T, eye], axis=1)
    cm16 = cm.astype(ml_dtypes.bfloat16)
    bias_col = np.full(
        (H, 1), np.asarray(bias, np.float32).reshape(-1)[0], np.float32
    )
    bias_bits = bias_col.view(np.uint16).view(ml_dtypes.bfloat16)  # [H, 2]
    return np.ascontiguousarray(np.concatenate([cm16, bias_bits], axis=1))


def kernel(x, K, bias, lambda_c, lambda_a, _trace=False):
    global LAST_RESULTS
    import ml_dtypes
    from concourse.bass_utils import run_bass_kernel_spmd

    x = np.asarray(x, np.float32)
    cm16 = _build_consts(K, bias, lambda_c, lambda_a)
    nc = _get_program()

    in_maps = []
    for core in range(N_CORES):
        shard = x[core * B_LOC : (core + 1) * B_LOC]  # [B_LOC, C, H, W]
        shard_t = np.ascontiguousarray(
            shard.transpose(2, 0, 1, 3).astype(ml_dtypes.bfloat16)
        )  # [H, B_LOC, C, W] bf16
        in_maps.append({"x": shard_t, "cmat": cm16})

    res = run_bass_kernel_spmd(
        nc, in_maps, core_ids=list(range(N_CORES)), trace=_trace
    )
    LAST_RESULTS = res
    out = np.concatenate(
        [r["out"].transpose(1, 0, 2) for r in res.results], axis=0
    )
    return out.reshape(B, 1, H, W).astype(np.float32, copy=False)


# revision 9
# speedup vs baseline: 1.1925x; 1.1925x over previous
"""CommutatorConv2d kernel for Trainium2 (Bass/Tile), 8-core data-parallel.

Math: the reference's commutator/anticommutator conv reduces exactly to a
single-channel 3x3 conv on the channel-summed input:

    out[b] = T @ xs[b] @ A + Bm @ xs[b] @ T + bias,   xs = x.sum(axis=1)

where T is the 128x128 tridiagonal-ones matrix and A, Bm are tridiagonal
matrices built from K's column/row sums scaled by (lambda_c +/- lambda_a).

v5 (all bf16; harness gate is 2e-2, this lands ~5e-3):
- One fused DRAM tensor per core: each partition row is
  [cmat row (A|T|BmT|I|bias-bits, 514 cols) | batch1 (32*128) | batch0].
  The constants ride inside the sync ring's FIRST piece, so they cost
  zero extra descriptors (HWDGE descriptor generation, ~15ns/desc/queue,
  is the stream bottleneck) and land before any compute needs them.
- Pieces of 16/12/4 channels per batch (4KB/3KB/1KB descriptors);
  batch 1 on the sync ring, batch 0 on the scalar ring.
- PE folds each batch's 16ch piece as 4 accumulating N=512 identity
  matmuls after a junk-matmul warmup block that ramps the PE out of its
  low/mid p-state; ACT evacuates the [128,512] stacks; the DVE (bf16 2x)
  trees the 12ch/4ch pieces, finishes the stacks, and combines into xs.
- Sandwich: uv = xs.T @ [T|BmT]; out = uv1.T @ A + uv2.T @ T in PSUM;
  ACT fuses the bias on PSUM evacuation. Stores are split into
  half-partition DMAs across both rings (halves descriptor-gen time).
"""

import numpy as np

B, C, H, W = 16, 32, 128, 128
N_CORES = 8
B_LOC = B // N_CORES

PIECE_CH = (16, 12, 4)
CMCOLS = 4 * W + 2
N_JUNK = 16

_PROGRAM = None
LAST_RESULTS = None


def _build_program():
    import concourse.mybir as mybir
    from concourse import bacc
    from concourse.bass import MemorySpace
    from concourse.tile import TileContext

    bf16 = mybir.dt.bfloat16
    f32 = mybir.dt.float32
    nc = bacc.Bacc(
        "TRN2", target_bir_lowering=False, debug=False, num_devices=N_CORES
    )

    ncols = CMCOLS + 2 * C * W
    xc_dram = nc.dram_tensor("xc", (H, ncols), bf16, kind="ExternalInput")
    out_dram = nc.dram_tensor("out", (H, B_LOC, W), f32, kind="ExternalOutput")

    xc_ap = xc_dram.ap()
    out_ap = out_dram.ap()

    def col(b, c):
        # start column of channel c of batch b in the fused tensor
        # (batch 1 first, then batch 0)
        return CMCOLS + (1 - b) * C * W + c * W

    with TileContext(nc) as tc:
        with (
            tc.tile_pool(name="xpool", bufs=1) as xpool,
            tc.tile_pool(name="spool", bufs=1) as spool,
            tc.tile_pool(name="psum", bufs=1, space=MemorySpace.PSUM) as ppool,
        ):
            # PE warmup scratch (zeroed on gpsimd, otherwise idle)
            scratch = spool.tile([H, 5 * W], bf16, tag="scratch")
            nc.gpsimd.memset(scratch, 0.0)

            # sync ring: [cmat + b1p0] then b1p1, b1p2, then store halves;
            # scalar ring: b0 pieces then store halves
            head = xpool.tile([H, CMCOLS + PIECE_CH[0] * W], bf16, tag="head")
            nc.sync.dma_start(out=head, in_=xc_ap[:, 0 : CMCOLS + PIECE_CH[0] * W])
            cm_sb = head[:, 0:CMCOLS]
            a_sb = cm_sb[:, 0:W]
            t_sb = cm_sb[:, W : 2 * W]
            tbm_sb = cm_sb[:, W : 3 * W]  # [T | BmT]
            i_sb = cm_sb[:, 3 * W : 4 * W]
            bias_sb = cm_sb[:, 4 * W : 4 * W + 2].bitcast(f32)

            tiles = {(1, 0): head[:, CMCOLS : CMCOLS + PIECE_CH[0] * W]}
            for b, eng in ((1, nc.sync), (0, nc.scalar)):
                c0 = 0
                for p, nch in enumerate(PIECE_CH):
                    if (b, p) not in tiles:
                        xq = xpool.tile([H, nch * W], bf16, tag=f"x{b}_{p}")
                        eng.dma_start(
                            out=xq, in_=xc_ap[:, col(b, c0) : col(b, c0 + nch)]
                        )
                        tiles[(b, p)] = xq
                    c0 += nch

            # ---- PE: warmup junk quads, then each batch's 16ch piece as
            # 4 accumulating N=512 identity matmuls ----
            junk_psum = ppool.tile([H, 4 * W], f32, tag="junk")
            for j in range(N_JUNK):
                nc.tensor.matmul(
                    junk_psum,
                    scratch[:, 0:W],
                    scratch[:, W : 5 * W],
                    start=True,
                    stop=True,
                    skip_group_check=True,
                )

            fold_psum = {}
            for b in (1, 0):
                psum = ppool.tile([H, 4 * W], f32, tag=f"fold{b}")
                xq = tiles[(b, 0)]
                for q in range(4):
                    nc.tensor.matmul(
                        psum,
                        i_sb,
                        xq[:, q * 4 * W : (q + 1) * 4 * W],
                        start=(q == 0),
                        stop=(q == 3),
                        skip_group_check=True,
                    )
                fold_psum[b] = psum

            # ACT evacuates fold PSUMs to bf16 (mid-stream, hidden)
            p0_sb = {}
            for b in (1, 0):
                sb = spool.tile([H, 4 * W], bf16, tag=f"p0_{b}")
                nc.scalar.copy(sb, fold_psum[b])
                p0_sb[b] = sb

            # ---- DVE (ready-order): 12ch trees, then per-batch finish +
            # 4ch tail tree + combines ----
            def tree(ap_tile, nelem):
                n = nelem
                while n > W and n % 2 == 0 and (n // 2) % W == 0:
                    n //= 2
                    nc.vector.tensor_add(
                        ap_tile[:, :n], ap_tile[:, :n], ap_tile[:, n : 2 * n]
                    )
                if n == 3 * W:
                    nc.vector.tensor_add(
                        ap_tile[:, :W], ap_tile[:, :W], ap_tile[:, W : 2 * W]
                    )
                    nc.vector.tensor_add(
                        ap_tile[:, :W], ap_tile[:, :W], ap_tile[:, 2 * W : 3 * W]
                    )

            xs = {}
            tree(tiles[(1, 1)], 12 * W)
            tree(tiles[(0, 1)], 12 * W)
            for b in (1, 0):
                pp = p0_sb[b]
                tree(pp, 4 * W)  # finish PE stack: 512 -> 128
                t4 = tiles[(b, 2)]
                tree(t4, 4 * W)
                nc.vector.tensor_add(pp[:, :W], pp[:, :W], tiles[(b, 1)][:, :W])
                nc.vector.tensor_add(pp[:, :W], pp[:, :W], t4[:, :W])
                xs[b] = pp[:, :W]

            # ---- sandwich per batch (b1 first); half-split stores ----
            for b in (1, 0):
                uv_psum = ppool.tile([H, 2 * W], f32, tag=f"uv{b}p")
                nc.tensor.matmul(uv_psum, xs[b], tbm_sb, start=True, stop=True)
                uv_sb = spool.tile([H, 2 * W], bf16, tag=f"uv{b}")
                nc.scalar.copy(uv_sb, uv_psum)
                o_psum = ppool.tile([H, W], f32, tag=f"o{b}p")
                nc.tensor.matmul(
                    o_psum, uv_sb[:, 0:W], a_sb, start=True, stop=False,
                    skip_group_check=True,
                )
                nc.tensor.matmul(
                    o_psum, uv_sb[:, W : 2 * W], t_sb, start=False, stop=True,
                    skip_group_check=True,
                )
                o_sb = spool.tile([H, W], f32, tag=f"o{b}")
                nc.scalar.add(o_sb, o_psum, add=bias_sb)
                nc.sync.dma_start(
                    out=out_ap[0 : H // 2, b, :], in_=o_sb[0 : H // 2, :]
                )
                nc.scalar.dma_start(
                    out=out_ap[H // 2 : H, b, :], in_=o_sb[H // 2 : H, :]
                )

    nc.compile()
    return nc


def _get_program():
    global _PROGRAM
    if _PROGRAM is None:
        _PROGRAM = _build_program()
    return _PROGRAM


def _build_consts(K, bias, lambda_c, lambda_a):
    import ml_dtypes

    K = np.asarray(K, np.float32)
    lc = float(np.asarray(lambda_c))
    la = float(np.asarray(lambda_a))
    a = (lc + la) * K.sum(axis=0)  # column sums -> horizontal taps
    b = (la - lc) * K.sum(axis=1)  # row sums -> vertical taps
    eye = np.eye(H, dtype=np.float32)
    up = np.eye(H, k=1, dtype=np.float32)
    dn = np.eye(H, k=-1, dtype=np.float32)
    T = eye + up + dn
    A = a[1] * eye + a[0] * up + a[2] * dn
    Bm = b[1] * eye + b[2] * up + b[0] * dn
    cm = np.concatenate([A, T, Bm.T, eye], axis=1)
    cm16 = cm.astype(ml_dtypes.bfloat16)
    bias_col = np.full(
        (H, 1), np.asarray(bias, np.float32).reshape(-1)[0], np.float32
    )
    bias_bits = bias_col.view(np.uint16).view(ml_dtypes.bfloat16)  # [H, 2]
    return np.concatenate([cm16, bias_bits], axis=1)


def kernel(x, K, bias, lambda_c, lambda_a, _trace=False):
    global LAST_RESULTS
    import ml_dtypes
    from concourse.bass_utils import run_bass_kernel_spmd

    x = np.asarray(x, np.float32)
    cmb = _build_consts(K, bias, lambda_c, lambda_a)
    nc = _get_program()

    in_maps = []
    for core in range(N_CORES):
        shard = x[core * B_LOC : (core + 1) * B_LOC]  # [B_LOC, C, H, W]
        shard_t = shard.transpose(2, 0, 1, 3).astype(ml_dtypes.bfloat16)
        # fused per-partition rows: [cmat | batch1 | batch0]
        xc = np.concatenate(
            [
                cmb,
                shard_t[:, 1].reshape(H, C * W),
                shard_t[:, 0].reshape(H, C * W),
            ],
            axis=1,
        )
        in_maps.append({"xc": np.ascontiguousarray(xc)})

    res = run_bass_kernel_spmd(
        nc, in_maps, core_ids=list(range(N_CORES)), trace=_trace
    )
    LAST_RESULTS = res
    out = np.concatenate(
        [r["out"].transpose(1, 0, 2) for r in res.results], axis=0
    )
    return out.reshape(B, 1, H, W).astype(np.float32, copy=False)


# revision 13
# speedup vs baseline: 1.2596x; 1.0562x over previous
"""CommutatorConv2d kernel for Trainium2 (Bass/Tile), 8-core data-parallel.

Math: the reference's commutator/anticommutator conv reduces exactly to a
single-channel 3x3 conv on the channel-summed input:

    out[b] = T @ xs[b] @ A + Bm @ xs[b] @ T + bias,   xs = x.sum(axis=1)

where T is the 128x128 tridiagonal-ones matrix and A, Bm are tridiagonal
matrices built from K's column/row sums scaled by (lambda_c +/- lambda_a).

v5 (all bf16; harness gate is 2e-2, this lands ~5e-3):
- One fused DRAM tensor per core: each partition row is
  [cmat row (A|T|BmT|I|bias-bits, 514 cols) | batch1 (32*128) | batch0].
  The constants ride inside the sync ring's FIRST piece, so they cost
  zero extra descriptors (HWDGE descriptor generation, ~15ns/desc/queue,
  is the stream bottleneck) and land before any compute needs them.
- Pieces of 16/12/4 channels per batch (4KB/3KB/1KB descriptors);
  batch 1 on the sync ring, batch 0 on the scalar ring.
- PE folds each batch's 16ch piece as 4 accumulating N=512 identity
  matmuls after a junk-matmul warmup block that ramps the PE out of its
  low/mid p-state; ACT evacuates the [128,512] stacks; the DVE (bf16 2x)
  trees the 12ch/4ch pieces, finishes the stacks, and combines into xs.
- Sandwich: uv = xs.T @ [T|BmT]; out = uv1.T @ A + uv2.T @ T in PSUM;
  ACT fuses the bias on PSUM evacuation. Stores are split into
  half-partition DMAs across both rings (halves descriptor-gen time).
"""

import numpy as np

B, C, H, W = 16, 32, 128, 128
N_CORES = 8
B_LOC = B // N_CORES

PIECE_B1 = (16, 12, 4)  # sync ring: head piece carries cmat too
PIECE_B0 = (24, 8)  # scalar ring: fewer DMAs (desc-gen is ~13.5ns/desc)
CMCOLS = 4 * W + 2
N_JUNK = 12

_PROGRAM = None
LAST_RESULTS = None


def _build_program():
    import concourse.mybir as mybir
    from concourse import bacc
    from concourse.bass import MemorySpace
    from concourse.tile import TileContext

    bf16 = mybir.dt.bfloat16
    f32 = mybir.dt.float32
    nc = bacc.Bacc(
        "TRN2", target_bir_lowering=False, debug=False, num_devices=N_CORES
    )

    ncols = CMCOLS + 2 * C * W
    xc_dram = nc.dram_tensor("xc", (H, ncols), bf16, kind="ExternalInput")
    out_dram = nc.dram_tensor("out", (H, B_LOC, W), f32, kind="ExternalOutput")

    xc_ap = xc_dram.ap()
    out_ap = out_dram.ap()

    def col(b, c):
        # start column of channel c of batch b in the fused tensor
        # (batch 1 first, then batch 0)
        return CMCOLS + (1 - b) * C * W + c * W

    with TileContext(nc) as tc:
        with (
            tc.tile_pool(name="xpool", bufs=1) as xpool,
            tc.tile_pool(name="spool", bufs=1) as spool,
            tc.tile_pool(name="psum", bufs=1, space=MemorySpace.PSUM) as ppool,
        ):
            # PE warmup scratch (zeroed on gpsimd, otherwise idle)
            scratch = spool.tile([H, 5 * W], bf16, tag="scratch")
            nc.gpsimd.memset(scratch, 0.0)

            # sync ring: [cmat + b1p0] then b1p1, b1p2, then store halves;
            # scalar ring: b0 pieces then store halves
            head = xpool.tile([H, CMCOLS + PIECE_B1[0] * W], bf16, tag="head")
            nc.sync.dma_start(out=head, in_=xc_ap[:, 0 : CMCOLS + PIECE_B1[0] * W])
            cm_sb = head[:, 0:CMCOLS]
            a_sb = cm_sb[:, 0:W]
            t_sb = cm_sb[:, W : 2 * W]
            tbm_sb = cm_sb[:, W : 3 * W]  # [T | BmT]
            i_sb = cm_sb[:, 3 * W : 4 * W]
            bias_sb = cm_sb[:, 4 * W : 4 * W + 2].bitcast(f32)

            tiles = {(1, 0): head[:, CMCOLS : CMCOLS + PIECE_B1[0] * W]}
            for b, eng, pieces in (
                (1, nc.sync, PIECE_B1),
                (0, nc.scalar, PIECE_B0),
            ):
                c0 = 0
                for p, nch in enumerate(pieces):
                    if (b, p) not in tiles:
                        xq = xpool.tile([H, nch * W], bf16, tag=f"x{b}_{p}")
                        eng.dma_start(
                            out=xq, in_=xc_ap[:, col(b, c0) : col(b, c0 + nch)]
                        )
                        tiles[(b, p)] = xq
                    c0 += nch

            # ---- PE: warmup junk quads, then each batch's 16ch piece as
            # 4 accumulating N=512 identity matmuls ----
            junk_psum = ppool.tile([H, 4 * W], f32, tag="junk")
            for j in range(N_JUNK):
                nc.tensor.matmul(
                    junk_psum,
                    scratch[:, 0:W],
                    scratch[:, W : 5 * W],
                    start=True,
                    stop=True,
                    skip_group_check=True,
                )

            fold_psum = {}
            for b, nq in ((1, PIECE_B1[0] // 4), (0, PIECE_B0[0] // 4)):
                psum = ppool.tile([H, 4 * W], f32, tag=f"fold{b}")
                xq = tiles[(b, 0)]
                for q in range(nq):
                    nc.tensor.matmul(
                        psum,
                        i_sb,
                        xq[:, q * 4 * W : (q + 1) * 4 * W],
                        start=(q == 0),
                        stop=(q == nq - 1),
                        skip_group_check=True,
                    )
                fold_psum[b] = psum

            # ACT evacuates fold PSUMs to bf16 (mid-stream, hidden)
            p0_sb = {}
            for b in (1, 0):
                sb = spool.tile([H, 4 * W], bf16, tag=f"p0_{b}")
                nc.scalar.copy(sb, fold_psum[b])
                p0_sb[b] = sb

            # ---- DVE (ready-order): 12ch trees, then per-batch finish +
            # 4ch tail tree + combines ----
            def tree(ap_tile, nelem):
                n = nelem
                while n > W and n % 2 == 0 and (n // 2) % W == 0:
                    n //= 2
                    nc.vector.tensor_add(
                        ap_tile[:, :n], ap_tile[:, :n], ap_tile[:, n : 2 * n]
                    )
                if n == 3 * W:
                    nc.vector.tensor_add(
                        ap_tile[:, :W], ap_tile[:, :W], ap_tile[:, W : 2 * W]
                    )
                    nc.vector.tensor_add(
                        ap_tile[:, :W], ap_tile[:, :W], ap_tile[:, 2 * W : 3 * W]
                    )

            xs = {}
            tree(tiles[(1, 1)], 12 * W)  # b1 12ch piece
            tree(tiles[(0, 1)], 8 * W)  # b0 8ch tail piece
            # b1: finish stack, tree the 4ch tail, combine
            pp = p0_sb[1]
            tree(pp, 4 * W)
            tree(tiles[(1, 2)], 4 * W)
            nc.vector.tensor_add(pp[:, :W], pp[:, :W], tiles[(1, 1)][:, :W])
            nc.vector.tensor_add(pp[:, :W], pp[:, :W], tiles[(1, 2)][:, :W])
            xs[1] = pp[:, :W]
            # b0: finish stack, combine with the 8ch tree partial
            pp = p0_sb[0]
            tree(pp, 4 * W)
            nc.vector.tensor_add(pp[:, :W], pp[:, :W], tiles[(0, 1)][:, :W])
            xs[0] = pp[:, :W]

            # ---- sandwich per batch (b1 first); half-split stores ----
            for b in (1, 0):
                uv_psum = ppool.tile([H, 2 * W], f32, tag=f"uv{b}p")
                nc.tensor.matmul(uv_psum, xs[b], tbm_sb, start=True, stop=True)
                uv_sb = spool.tile([H, 2 * W], bf16, tag=f"uv{b}")
                nc.scalar.copy(uv_sb, uv_psum)
                o_psum = ppool.tile([H, W], f32, tag=f"o{b}p")
                nc.tensor.matmul(
                    o_psum, uv_sb[:, 0:W], a_sb, start=True, stop=False,
                    skip_group_check=True,
                )
                nc.tensor.matmul(
                    o_psum, uv_sb[:, W : 2 * W], t_sb, start=False, stop=True,
                    skip_group_check=True,
                )
                o_sb = spool.tile([H, W], f32, tag=f"o{b}")
                nc.scalar.add(o_sb, o_psum, add=bias_sb)
                nc.sync.dma_start(
                    out=out_ap[0 : H // 2, b, :], in_=o_sb[0 : H // 2, :]
                )
                nc.scalar.dma_start(
                    out=out_ap[H // 2 : H, b, :], in_=o_sb[H // 2 : H, :]
                )

    nc.compile()
    return nc


def _get_program():
    global _PROGRAM
    if _PROGRAM is None:
        _PROGRAM = _build_program()
    return _PROGRAM


def _build_consts(K, bias, lambda_c, lambda_a):
    import ml_dtypes

    K = np.asarray(K, np.float32)
    lc = float(np.asarray(lambda_c))
    la = float(np.asarray(lambda_a))
    a = (lc + la) * K.sum(axis=0)  # column sums -> horizontal taps
    b = (la - lc) * K.sum(axis=1)  # row sums -> vertical taps
    eye = np.eye(H, dtype=np.float32)
    up = np.eye(H, k=1, dtype=np.float32)
    dn = np.eye(H, k=-1, dtype=np.float32)
    T = eye + up + dn
    A = a[1] * eye + a[0] * up + a[2] * dn
    Bm = b[1] * eye + b[2] * up + b[0] * dn
    cm = np.concatenate([A, T, Bm.T, eye], axis=1)
    cm16 = cm.astype(ml_dtypes.bfloat16)
    bias_col = np.full(
        (H, 1), np.asarray(bias, np.float32).reshape(-1)[0], np.float32
    )
    bias_bits = bias_col.view(np.uint16).view(ml_dtypes.bfloat16)  # [H, 2]
    return np.concatenate([cm16, bias_bits], axis=1)


def kernel(x, K, bias, lambda_c, lambda_a, _trace=False):
    global LAST_RESULTS
    import ml_dtypes
    from concourse.bass_utils import run_bass_kernel_spmd

    x = np.asarray(x, np.float32)
    cmb = _build_consts(K, bias, lambda_c, lambda_a)
    nc = _get_program()

    in_maps = []
    for core in range(N_CORES):
        shard = x[core * B_LOC : (core + 1) * B_LOC]  # [B_LOC, C, H, W]
        shard_t = shard.transpose(2, 0, 1, 3).astype(ml_dtypes.bfloat16)
        # fused per-partition rows: [cmat | batch1 | batch0]
        xc = np.concatenate(
            [
                cmb,
                shard_t[:, 1].reshape(H, C * W),
                shard_t[:, 0].reshape(H, C * W),
            ],
            axis=1,
        )
        in_maps.append({"xc": np.ascontiguousarray(xc)})

    res = run_bass_kernel_spmd(
        nc, in_maps, core_ids=list(range(N_CORES)), trace=_trace
    )
    LAST_RESULTS = res
    out = np.concatenate(
        [r["out"].transpose(1, 0, 2) for r in res.results], axis=0
    )
    return out.reshape(B, 1, H, W).astype(np.float32, copy=False)
